# revision 4
# baseline (speedup 1.0000x reference)
"""Trainium2 Bass kernel for nn_ConvertParamsLayer_16277926052102 (dense_mlp).

Math per sample b (B=4096, H=64, V=256):
    scale[h] = sqrt(cd1[h] / cd2[h])
    wt2[h,v] = scale[h] * wt1[h,v]
    b2[v]    = b1[v] + sum_h wt1[h,v] * (muh1[h] - scale[h]*muh2[h])

Sharding: pure data parallel over B across 8 cores (512 samples each).

Per-core layout: samples are processed in pairs; an SBUF tile holds
[128 partitions = (sample_in_pair, h), free = v].  The wt2 scaling is a
per-partition tensor_scalar multiply.  The b2 reduction over h is a PE
matmul with the wt1 v-half slice as the stationary operand and a
[128, 2] block coefficient matrix as the moving operand (col 0 = sample
A coeffs on partitions 0:64, col 1 = sample B coeffs on 64:128), so the
PSUM output is [v, sample] packed along the free dim; a block of 64
pairs is transposed back to sample-major at evacuation time.
"""

import functools

import numpy as np

import concourse.bacc as bacc
import concourse.mybir as mybir
import concourse.tile as tile
from concourse._compat import with_exitstack
from concourse.bass_utils import run_bass_kernel_spmd

N_CORES = 8
B, H, V = 4096, 64, 256
BL = B // N_CORES           # 512 samples per core
PAIRS = BL // 2             # 256 sample pairs per core
G = 8                       # pairs per DMA group (1 MiB transfers)
T = PAIRS // G              # 32 groups
BLK = 64                    # pairs per b2 PSUM block (128 samples)
GPB = BLK // G              # groups per b2 block
F32 = mybir.dt.float32

# which in-group pair indices run their wt2 multiply on ScalarE (rest on DVE)
ACT_G = (2, 5)

# set by test.py to get profiling info back
TRACE = False
LAST_RESULTS = None


@with_exitstack
def _body(ctx, tc, nc, wt1, b1, muh1, muh2, cd1, cd2, wt2, b2):
    f32 = F32
    Act = mybir.ActivationFunctionType

    const = ctx.enter_context(tc.tile_pool(name="const", bufs=1))
    win_pool = ctx.enter_context(tc.tile_pool(name="win_pool", bufs=3))
    wout_pool = ctx.enter_context(tc.tile_pool(name="wout_pool", bufs=3))
    bpool = ctx.enter_context(tc.tile_pool(name="bpool", bufs=2))
    pt_pool = ctx.enter_context(tc.tile_pool(name="pt_pool", bufs=2, space="PSUM"))
    pb_pool = ctx.enter_context(tc.tile_pool(name="pb_pool", bufs=2, space="PSUM"))

    ident_dram = nc.inline_tensor(np.eye(128, dtype=np.float32), name="ident128")
    ident = const.tile([128, 128], f32)
    nc.sync.dma_start(ident[:], ident_dram.ap())

    # ---- small [512, 64] tensors in "Z" layout: Z[r, (j, q)] = x_flat[j*16384 + r*128 + q].
    # Column p = 128j + r of the transposed view is the pair-p column
    # [x[2p, :]; x[2p+1, :]] over partitions (sample_in_pair, h).
    def zload(t, nm):
        tl = const.tile([128, 256], f32, name=nm)
        nc.sync.dma_start(
            tl.rearrange("p (j q) -> p j q", j=2),
            t.ap().rearrange("(j r i) h -> r j (i h)", j=2, r=128, i=2),
        )
        return tl

    cd1_z = zload(cd1, "cd1_z")
    cd2_z = zload(cd2, "cd2_z")
    muh1_z = zload(muh1, "muh1_z")
    muh2_z = zload(muh2, "muh2_z")

    rec = const.tile([128, 256], f32)
    nc.vector.reciprocal(rec[:], cd2_z[:])
    rat = const.tile([128, 256], f32)
    nc.vector.tensor_mul(rat[:], cd1_z[:], rec[:])
    scale_z = const.tile([128, 256], f32)
    nc.scalar.activation(scale_z[:], rat[:], Act.Sqrt)
    smu2 = const.tile([128, 256], f32)
    nc.vector.tensor_mul(smu2[:], scale_z[:], muh2_z[:])
    cc_z = const.tile([128, 256], f32)
    nc.vector.tensor_sub(cc_z[:], muh1_z[:], smu2[:])

    # ---- transpose to pair-column layouts.
    # scale_p[:, p] = [scale[2p, :]; scale[2p+1, :]]  (dense, for tensor_scalar)
    # cc_split[:, 2p] = [cc[2p, :]; 0], cc_split[:, 2p+1] = [0; cc[2p+1, :]]  (matmul lhsT)
    scale_p = const.tile([128, 256], f32)
    cc_split = const.tile([128, 512], f32)
    nc.vector.memset(cc_split[:], 0.0)
    cs_view = cc_split.rearrange("k (p two) -> k two p", two=2)
    for j in range(2):
        tp_s = pt_pool.tile([128, 128], f32, name="tp_s", tag="ptile")
        nc.tensor.transpose(tp_s[:], scale_z[:, 128 * j:128 * (j + 1)], ident[:])
        nc.vector.tensor_copy(scale_p[:, 128 * j:128 * (j + 1)], tp_s[:])
        tp_c = pt_pool.tile([128, 128], f32, name="tp_c", tag="ptile")
        nc.tensor.transpose(tp_c[:], cc_z[:, 128 * j:128 * (j + 1)], ident[:])
        nc.vector.tensor_copy(cs_view[0:64, 0, 128 * j:128 * (j + 1)], tp_c[0:64, :])
        nc.vector.tensor_copy(cs_view[64:128, 1, 128 * j:128 * (j + 1)], tp_c[64:128, :])

    # ---- main loop over pair groups ----
    wt1_v = wt1.ap().rearrange("(t g i) h v -> (i h) t g v", t=T, g=G, i=2)
    wt2_v = wt2.ap().rearrange("(t g i) h v -> (i h) t g v", t=T, g=G, i=2)
    b1_v = b1.ap().rearrange("(c l) v -> l c v", l=128)
    b2_v = b2.ap().rearrange("(c l) v -> l c v", l=128)

    pbv = [None, None]
    b1_t = None
    for t in range(T):
        if t % GPB == 0:
            blk = t // GPB
            pbv[0] = pb_pool.tile([128, 2 * BLK], f32, name="pbv0")
            pbv[1] = pb_pool.tile([128, 2 * BLK], f32, name="pbv1")
            b1_t = bpool.tile([128, V], f32, name="b1_t")
            nc.scalar.dma_start(b1_t[:], b1_v[:, blk, :])
        win = win_pool.tile([128, G * V], f32, name="win")
        nc.sync.dma_start(win.rearrange("p (g v) -> p g v", g=G), wt1_v[:, t, :, :])
        wout = wout_pool.tile([128, G * V], f32, name="wout")
        for g in range(G):
            p = t * G + g
            pl = p % BLK
            src = win[:, g * V:(g + 1) * V]
            dst = wout[:, g * V:(g + 1) * V]
            if g in ACT_G:
                nc.scalar.mul(dst, src, scale_p[:, p:p + 1])
            else:
                nc.vector.tensor_scalar_mul(dst, src, scale_p[:, p:p + 1])
            for jv in range(2):
                nc.tensor.matmul(
                    pbv[jv][:, 2 * pl:2 * pl + 2],
                    win[:, g * V + 128 * jv:g * V + 128 * (jv + 1)],
                    cc_split[:, 2 * p:2 * p + 2],
                    start=True,
                    stop=True,
                )
        nc.gpsimd.dma_start(wt2_v[:, t, :, :], wout.rearrange("p (g v) -> p g v", g=G))
        if t % GPB == GPB - 1:
            blk = t // GPB
            b2_sb = bpool.tile([128, V], f32, name="b2_sb")
            for jv in range(2):
                tb = bpool.tile([128, 2 * BLK], f32, name="tb")
                nc.vector.tensor_copy(tb[:], pbv[jv][:])
                pt = pt_pool.tile([128, 2 * BLK], f32, name="pt", tag="ptile")
                nc.tensor.transpose(pt[:], tb[:], ident[:])
                nc.vector.tensor_add(
                    b2_sb[:, 128 * jv:128 * (jv + 1)],
                    pt[:],
                    b1_t[:, 128 * jv:128 * (jv + 1)],
                )
            nc.scalar.dma_start(b2_v[:, blk, :], b2_sb[:])


@functools.lru_cache(maxsize=1)
def _program():
    nc = bacc.Bacc(
        "TRN2", target_bir_lowering=False, debug=False, num_devices=N_CORES
    )
    wt1 = nc.dram_tensor("wt1", [BL, H, V], F32, kind="ExternalInput")
    b1 = nc.dram_tensor("b1", [BL, V], F32, kind="ExternalInput")
    muh1 = nc.dram_tensor("muh1", [BL, H], F32, kind="ExternalInput")
    muh2 = nc.dram_tensor("muh2", [BL, H], F32, kind="ExternalInput")
    cd1 = nc.dram_tensor("covh_diag1", [BL, H], F32, kind="ExternalInput")
    cd2 = nc.dram_tensor("covh_diag2", [BL, H], F32, kind="ExternalInput")
    wt2 = nc.dram_tensor("wt2", [BL, H, V], F32, kind="ExternalOutput")
    b2 = nc.dram_tensor("b2", [BL, V], F32, kind="ExternalOutput")
    with tile.TileContext(nc) as tc:
        _body(tc, nc, wt1, b1, muh1, muh2, cd1, cd2, wt2, b2)
    nc.compile()
    return nc


def kernel(b1, wt1, muh1, muh2, covh_diag1, covh_diag2):
    global LAST_RESULTS
    nc = _program()
    in_maps = []
    for m in range(N_CORES):
        sl = slice(m * BL, (m + 1) * BL)
        in_maps.append({
            "wt1": np.ascontiguousarray(wt1[sl]),
            "b1": np.ascontiguousarray(b1[sl]),
            "muh1": np.ascontiguousarray(muh1[sl]),
            "muh2": np.ascontiguousarray(muh2[sl]),
            "covh_diag1": np.ascontiguousarray(covh_diag1[sl]),
            "covh_diag2": np.ascontiguousarray(covh_diag2[sl]),
        })
    res = run_bass_kernel_spmd(
        nc, in_maps, core_ids=list(range(N_CORES)), trace=TRACE
    )
    LAST_RESULTS = res
    b2_full = np.concatenate([r["b2"] for r in res.results], axis=0)
    wt2_full = np.concatenate([r["wt2"] for r in res.results], axis=0)
    return b2_full, wt2_full


# revision 8
# speedup vs baseline: 4.5951x; 4.5951x over previous
"""Trainium2 Bass kernel for nn_ConvertParamsLayer_16277926052102 (dense_mlp).

Math per sample b (B=4096, H=64, V=256):
    scale[h] = sqrt(cd1[h] / cd2[h])
    wt2[h,v] = scale[h] * wt1[h,v]
    b2[v]    = b1[v] + sum_h wt1[h,v] * (muh1[h] - scale[h]*muh2[h])

Sharding: pure data parallel over B across 8 cores (512 samples each).

Per-core layout: samples are processed in pairs; an SBUF tile holds
[128 partitions = (sample_in_pair, h), free = v].  The wt2 scaling is a
per-partition tensor_scalar multiply.  The b2 reduction over h is a PE
matmul with the wt1 v-half slice as the stationary operand and a
[128, 2] block coefficient matrix as the moving operand (col 0 = sample
A coeffs on partitions 0:64, col 1 = sample B coeffs on 64:128), so the
PSUM output is [v, sample] packed along the free dim; a block of 64
pairs is transposed back to sample-major at evacuation time.
"""

import functools

import numpy as np

import concourse.bacc as bacc
import concourse.mybir as mybir
import concourse.tile as tile
from concourse._compat import with_exitstack
from concourse.bass_utils import run_bass_kernel_spmd

N_CORES = 8
B, H, V = 4096, 64, 256
BL = B // N_CORES           # 512 samples per core
PAIRS = BL // 2             # 256 sample pairs per core
G = 64                      # pairs per DMA chunk (8 MiB transfers)
T = PAIRS // G              # 4 chunks
BLK = 64                    # pairs per b2 PSUM block (128 samples) == G
F32 = mybir.dt.float32

# set by test.py to get profiling info back
TRACE = False
LAST_RESULTS = None


@with_exitstack
def _body(ctx, tc, nc, wt1, b1, muh1, muh2, cd1, cd2, wt2, b2):
    f32 = F32
    Act = mybir.ActivationFunctionType

    const = ctx.enter_context(tc.tile_pool(name="const", bufs=1))
    win_pool = ctx.enter_context(tc.tile_pool(name="win_pool", bufs=2))
    bpool = ctx.enter_context(tc.tile_pool(name="bpool", bufs=2))
    pt_pool = ctx.enter_context(tc.tile_pool(name="pt_pool", bufs=2, space="PSUM"))
    pb_pool = ctx.enter_context(tc.tile_pool(name="pb_pool", bufs=2, space="PSUM"))

    ident_dram = nc.inline_tensor(np.eye(128, dtype=np.float32), name="ident128")
    ident = const.tile([128, 128], f32)
    nc.scalar.dma_start(ident[:], ident_dram.ap())

    # ---- small [512, 64] tensors in "Z" layout: Z[r, (j, q)] = x_flat[j*16384 + r*128 + q].
    # Column p = 128j + r of the transposed view is the pair-p column
    # [x[2p, :]; x[2p+1, :]] over partitions (sample_in_pair, h).
    def zload(t, nm):
        tl = const.tile([128, 256], f32, name=nm)
        nc.scalar.dma_start(
            tl.rearrange("p (j q) -> p j q", j=2),
            t.ap().rearrange("(j r i) h -> r j (i h)", j=2, r=128, i=2),
        )
        return tl

    cd1_z = zload(cd1, "cd1_z")
    cd2_z = zload(cd2, "cd2_z")
    muh1_z = zload(muh1, "muh1_z")
    muh2_z = zload(muh2, "muh2_z")

    rec = const.tile([128, 256], f32)
    nc.vector.reciprocal(rec[:], cd2_z[:])
    rat = const.tile([128, 256], f32)
    nc.vector.tensor_mul(rat[:], cd1_z[:], rec[:])
    scale_z = const.tile([128, 256], f32)
    nc.scalar.activation(scale_z[:], rat[:], Act.Sqrt)
    smu2 = const.tile([128, 256], f32)
    nc.vector.tensor_mul(smu2[:], scale_z[:], muh2_z[:])
    cc_z = const.tile([128, 256], f32)
    nc.vector.tensor_sub(cc_z[:], muh1_z[:], smu2[:])

    # ---- transpose to pair-column layouts.
    # scale_p[:, p] = [scale[2p, :]; scale[2p+1, :]]  (dense, for tensor_scalar)
    # cc_split[:, 2p] = [cc[2p, :]; 0], cc_split[:, 2p+1] = [0; cc[2p+1, :]]  (matmul lhsT)
    scale_p = const.tile([128, 256], f32)
    cc_split = const.tile([128, 512], f32)
    nc.vector.memset(cc_split[:], 0.0)
    cs_view = cc_split.rearrange("k (p two) -> k two p", two=2)
    for j in range(2):
        tp_s = pt_pool.tile([128, 128], f32, name="tp_s", tag="ptile")
        nc.tensor.transpose(tp_s[:], scale_z[:, 128 * j:128 * (j + 1)], ident[:])
        nc.vector.tensor_copy(scale_p[:, 128 * j:128 * (j + 1)], tp_s[:])
        tp_c = pt_pool.tile([128, 128], f32, name="tp_c", tag="ptile")
        nc.tensor.transpose(tp_c[:], cc_z[:, 128 * j:128 * (j + 1)], ident[:])
        nc.vector.tensor_copy(cs_view[0:64, 0, 128 * j:128 * (j + 1)], tp_c[0:64, :])
        nc.vector.tensor_copy(cs_view[64:128, 1, 128 * j:128 * (j + 1)], tp_c[64:128, :])

    # ---- main loop over 4 chunks of 64 pairs (8 MiB each) ----
    # Loads and stores are interleaved across the two independent DMA
    # streams (sync = HWDGE, gpsimd = SWDGE): chunk c loads on stream c%2
    # and stores on stream (c+1)%2, so each stream carries 2 loads + 2
    # stores and the store of chunk c overlaps the load of chunk c+1.
    # The wt2 multiply is done in place (after the matmuls read the raw
    # wt1 values), halving SBUF so two 8 MiB chunk tiles fit.
    wt1_v = wt1.ap().rearrange("(t g i) h v -> (i h) t g v", t=T, g=G, i=2)
    wt2_v = wt2.ap().rearrange("(t g i) h v -> (i h) t g v", t=T, g=G, i=2)
    b1_v = b1.ap().rearrange("(c l) v -> l c v", l=128)
    b2_v = b2.ap().rearrange("(c l) v -> l c v", l=128)

    b1_all = const.tile([128, T * V], f32)
    nc.scalar.dma_start(b1_all.rearrange("p (c v) -> p c v", c=T), b1_v[:, :, :])
    b2_all = const.tile([128, T * V], f32)

    streams = (nc.sync, nc.gpsimd)
    for t in range(T):
        pbv = [
            pb_pool.tile([128, 2 * BLK], f32, name="pbv0"),
            pb_pool.tile([128, 2 * BLK], f32, name="pbv1"),
        ]
        win = win_pool.tile([128, G * V], f32, name="win")
        streams[t % 2].dma_start(
            win.rearrange("p (g v) -> p g v", g=G), wt1_v[:, t, :, :]
        )
        for g in range(G):
            p = t * G + g
            pl = p % BLK
            src = win[:, g * V:(g + 1) * V]
            for jv in range(2):
                nc.tensor.matmul(
                    pbv[jv][:, 2 * pl:2 * pl + 2],
                    win[:, g * V + 128 * jv:g * V + 128 * (jv + 1)],
                    cc_split[:, 2 * p:2 * p + 2],
                    start=True,
                    stop=True,
                )
            # in-place wt2 scaling (WAR on the matmuls above)
            nc.vector.tensor_scalar_mul(src, src, scale_p[:, p:p + 1])
        streams[(t + 1) % 2].dma_start(
            wt2_v[:, t, :, :], win.rearrange("p (g v) -> p g v", g=G)
        )
        # b2 block evacuation: [v, sample] PSUM -> transpose -> + b1
        for jv in range(2):
            tb = bpool.tile([128, 2 * BLK], f32, name="tb")
            nc.vector.tensor_copy(tb[:], pbv[jv][:])
            pt = pt_pool.tile([128, 2 * BLK], f32, name="pt", tag="ptile")
            nc.tensor.transpose(pt[:], tb[:], ident[:])
            nc.vector.tensor_add(
                b2_all[:, t * V + 128 * jv:t * V + 128 * (jv + 1)],
                pt[:],
                b1_all[:, t * V + 128 * jv:t * V + 128 * (jv + 1)],
            )
    nc.scalar.dma_start(b2_v[:, :, :], b2_all.rearrange("p (c v) -> p c v", c=T))


@functools.lru_cache(maxsize=1)
def _program():
    nc = bacc.Bacc(
        "TRN2", target_bir_lowering=False, debug=False, num_devices=N_CORES
    )
    wt1 = nc.dram_tensor("wt1", [BL, H, V], F32, kind="ExternalInput")
    b1 = nc.dram_tensor("b1", [BL, V], F32, kind="ExternalInput")
    muh1 = nc.dram_tensor("muh1", [BL, H], F32, kind="ExternalInput")
    muh2 = nc.dram_tensor("muh2", [BL, H], F32, kind="ExternalInput")
    cd1 = nc.dram_tensor("covh_diag1", [BL, H], F32, kind="ExternalInput")
    cd2 = nc.dram_tensor("covh_diag2", [BL, H], F32, kind="ExternalInput")
    wt2 = nc.dram_tensor("wt2", [BL, H, V], F32, kind="ExternalOutput")
    b2 = nc.dram_tensor("b2", [BL, V], F32, kind="ExternalOutput")
    with tile.TileContext(nc) as tc:
        _body(tc, nc, wt1, b1, muh1, muh2, cd1, cd2, wt2, b2)
    nc.compile()
    return nc


def kernel(b1, wt1, muh1, muh2, covh_diag1, covh_diag2):
    global LAST_RESULTS
    nc = _program()
    in_maps = []
    for m in range(N_CORES):
        sl = slice(m * BL, (m + 1) * BL)
        in_maps.append({
            "wt1": np.ascontiguousarray(wt1[sl]),
            "b1": np.ascontiguousarray(b1[sl]),
            "muh1": np.ascontiguousarray(muh1[sl]),
            "muh2": np.ascontiguousarray(muh2[sl]),
            "covh_diag1": np.ascontiguousarray(covh_diag1[sl]),
            "covh_diag2": np.ascontiguousarray(covh_diag2[sl]),
        })
    res = run_bass_kernel_spmd(
        nc, in_maps, core_ids=list(range(N_CORES)), trace=TRACE
    )
    LAST_RESULTS = res
    b2_full = np.concatenate([r["b2"] for r in res.results], axis=0)
    wt2_full = np.concatenate([r["wt2"] for r in res.results], axis=0)
    return b2_full, wt2_full
